# revision 1
# baseline (speedup 1.0000x reference)
"""Chamfer distance kernel for Trainium2 (Bass/Tile), 8 NeuronCores.

Full inputs: xyz1 [8, 4096, 3] f32, xyz2 [8, 4096, 3] f32.
Output: scalar f32 = mean(min_m d2[b,n,m]) + mean(min_n d2[b,n,m]).

Sharding: data-parallel over batch B=8, one batch element per core.
Each core computes partial sums [sum(dist1), sum(dist2)]; host combines
(and negates: the device works on -d2 throughout, see below).

Per-core algorithm: the NEGATED d2 matrix is produced directly by a K=9
f32r matmul (f32r = fp32 bits, 4x faster PE streaming) over augmented
operands that are host-side layouts of the input points:
  aug1 rows = [x1_d (3), x1_d^2 (3), 1 (3)]      (lhsT, [9, 4096])
  aug2 rows = [2*x2_d (3), -1 (3), -x2_d^2 (3)]  (rhs,  [9, 4096])
  psum[n, m] = -(||x1_n||^2 - 2 x1_n.x2_m + ||x2_m||^2) = -d2
Everything downstream is a MAX fold (min of d2 == max of -d2), which is
what the engines support best. Per [128 x 4096] row-block (n-chunk):
  - PE writes two [128 x 2048] PSUM tiles (4 matmuls each, one per bank);
  - ACT downcasts both tiles into a bf16 staging buffer (SBUF).
DVE (the bottleneck engine) then processes staging in GROUPS of up to 4
row-blocks so each instruction is maximally wide (per-op overhead is
~250ns): a 4-level 2x-mode bf16 tensor_tensor fold pyramid shrinks the
1x-mode rowmax reduce to 1/16 width (dist1), and a pairwise-tree + one
running max per group accumulates the dist2 plane R[128, 4096].
Progressive group sizes [2,2,4...] shorten the pipeline-fill ramp.
Epilogue: GPSIMD partition_all_reduce(max) folds R across partitions in
quarter-planes (each starting as soon as the last group's quarter-fold
lands, overlapping the DVE row-sums); dist1's partition-sum uses a tiny
PE ones-matmul.
Cost-model timeline: ~173 us/core total; DVE ~156 us busy, ACT ~136 us,
PE ~70 us. Reduce ops have no DVE perf modes (always 1 elem/lane/cycle),
which is why 2x-mode tensor_tensor prefolds carry most of the fold work.
"""

import numpy as np
from contextlib import ExitStack

import concourse.bass as bass
import concourse.bacc as bacc
import concourse.bass_isa as bass_isa
import concourse.mybir as mybir
from concourse.tile import TileContext
from concourse.bass_utils import run_bass_kernel_spmd

B, N, M, D = 8, 4096, 4096, 3
P = 128            # partitions (n-chunk size)
NI = N // P        # 32 n-chunks
FREE = 2048        # columns per DVE op (4 PSUM banks)
NJ = M // FREE     # 2 column groups
MM = 512           # matmul free dim (1 PSUM bank)
FDT = mybir.dt.float32
FRT = mybir.dt.float32r   # same bits as f32; PE streams 1 row/cycle (vs 4)
BDT = mybir.dt.bfloat16
AX = mybir.AxisListType
MAX = mybir.AluOpType.max
ADD = mybir.AluOpType.add

_CACHE = {}


def _build():
    nc = bacc.Bacc(None, target_bir_lowering=False)
    a1 = nc.dram_tensor("aug1", [9, N], FRT, kind="ExternalInput")
    a2 = nc.dram_tensor("aug2", [9, M], FRT, kind="ExternalInput")
    part = nc.dram_tensor("partial", [1, 2], FDT, kind="ExternalOutput")

    with ExitStack() as ctx:
        tc = ctx.enter_context(TileContext(nc))
        sb = ctx.enter_context(tc.tile_pool(name="sb", bufs=1))
        stg = ctx.enter_context(tc.tile_pool(name="stg", bufs=2))
        stf = ctx.enter_context(tc.tile_pool(name="stf", bufs=1))
        pp = ctx.enter_context(tc.tile_pool(name="pp", bufs=2, space="PSUM"))

        aug1 = sb.tile([9, N], FRT)
        aug2 = sb.tile([9, M], FRT)
        nc.sync.dma_start(out=aug1[:, :], in_=a1[:, :])
        nc.sync.dma_start(out=aug2[:, :], in_=a2[:, :])

        R = sb.tile([P, M], BDT)           # running max over n-chunks, per m
        D1 = sb.tile([P, NI], FDT)         # full-row max per n-chunk

        # ---- main loop ----
        # Per tile: PE matmul (f32r) -> PSUM; ACT downcasts PSUM -> bf16
        # staging. DVE (the bottleneck) runs everything in bf16, batched
        # over groups of G n-chunks so each instruction is as wide as
        # possible (DVE op overhead ~250ns): a 4-level 2x-mode fold
        # pyramid shrinks the 1x-mode rowmax-reduce to 1/16 width, and a
        # pairwise tree + one running max folds the dist2 plane.
        # Progressive group sizes: DVE starts working after one chunk's
        # copies instead of idling through a full 4-chunk group.
        GROUPS = [2, 2] + [4] * 7
        assert sum(GROUPS) == NI
        # fold-pyramid depth per group size (deeper batching amortizes the
        # ~250ns DVE per-op overhead)
        LEVELS = {1: 4, 2: 5, 4: 6}

        i0 = 0
        for gidx, G in enumerate(GROUPS):
            st = stg.tile([P, G, M], BDT, tag="st")
            for c in range(G):
                i = i0 + c
                for jh in range(NJ):
                    pt = pp.tile([P, FREE], FDT, tag="pt")
                    for k in range(FREE // MM):
                        nc.tensor.matmul(
                            pt[:, k * MM:(k + 1) * MM],
                            aug1[:, i * P:(i + 1) * P],
                            aug2[:, jh * FREE + k * MM: jh * FREE + (k + 1) * MM],
                            start=True, stop=True,
                        )
                    nc.scalar.copy(
                        st[:, c, jh * FREE:(jh + 1) * FREE], pt[:, :],
                    )
            # dist1 batched fold pyramid (per-chunk row maxes kept separate)
            prev, w = st, M
            for lvl in range(LEVELS[G]):
                nxt = stf.tile([P, G, w // 2], BDT, tag=f"f{lvl}")
                nc.vector.tensor_tensor(
                    out=nxt[:, :, :], in0=prev[:, :, 0:w // 2],
                    in1=prev[:, :, w // 2:w], op=MAX,
                )
                prev, w = nxt, w // 2
            nc.vector.tensor_reduce(
                out=D1[:, i0:i0 + G], in_=prev[:, :, :], axis=AX.X, op=MAX,
            )
            # dist2 pairwise tree within the group, then fold into R
            cur, width = st, G
            while width > 1:
                cv = cur[:, :, :].rearrange("p (a b) m -> p a b m", a=width // 2)
                nxt = stf.tile([P, width // 2, M], BDT, tag=f"t{width}")
                nc.vector.tensor_tensor(
                    out=nxt[:, :, :], in0=cv[:, :, 0, :], in1=cv[:, :, 1, :], op=MAX,
                )
                cur, width = nxt, width // 2
            t2 = cur[:, 0, :]
            if gidx == 0:
                nc.vector.tensor_copy(out=R[:, :], in_=t2)
            elif gidx == len(GROUPS) - 1:
                # last group: fold in quarter-planes so the epilogue's
                # GPSIMD partition folds can start before the full plane
                # is finished
                q = M // 4
                for qq in range(4):
                    nc.vector.tensor_tensor(
                        out=R[:, qq * q:(qq + 1) * q], in0=t2[:, qq * q:(qq + 1) * q],
                        in1=R[:, qq * q:(qq + 1) * q], op=MAX,
                    )
            else:
                nc.vector.tensor_tensor(out=R[:, :], in0=t2, in1=R[:, :], op=MAX)
            i0 += G

        # ---- dist1 epilogue: sum over i, partition-sum via PE ----
        s1 = sb.tile([P, 1], FDT)
        nc.vector.tensor_reduce(out=s1[:, :], in_=D1[:, :], axis=AX.X, op=ADD)
        ones_col = sb.tile([P, 1], FDT)
        nc.vector.memset(ones_col[:, :], 1.0)
        out_t = sb.tile([1, 2], FDT)

        p1 = pp.tile([1, 1], FDT, tag="pt")
        nc.tensor.matmul(p1[:, :], s1[:, :], ones_col[:, :], start=True, stop=True)
        nc.vector.tensor_copy(out=out_t[0:1, 0:1], in_=p1[0:1, 0:1])

        # ---- dist2 epilogue: GPSIMD partition fold, then row-sum ----
        # quarter-planes: each Pool fold starts as soon as its quarter of
        # R is final, and each DVE row-sum overlaps the next Pool fold
        Rr = sb.tile([P, M], BDT)
        q = M // 4
        s2h = sb.tile([1, 4], FDT)
        for qq in range(4):
            nc.gpsimd.partition_all_reduce(
                Rr[:, qq * q:(qq + 1) * q], R[:, qq * q:(qq + 1) * q],
                P, bass_isa.ReduceOp.max,
            )
        for qq in range(4):
            nc.vector.tensor_reduce(
                out=s2h[0:1, qq:qq + 1], in_=Rr[0:1, qq * q:(qq + 1) * q],
                axis=AX.X, op=ADD,
            )
        nc.vector.tensor_reduce(
            out=out_t[0:1, 1:2], in_=s2h[0:1, :], axis=AX.X, op=ADD,
        )

        nc.sync.dma_start(out=part[:, :], in_=out_t[0:1, :])

    nc.compile()
    return nc


def _get_nc():
    if "nc" not in _CACHE:
        _CACHE["nc"] = _build()
    return _CACHE["nc"]


def _augment(xyz1_b, xyz2_b):
    """Host-side layout of one batch element into the augmented operands."""
    a1 = np.empty((9, N), dtype=np.float32)
    t1 = xyz1_b.T.astype(np.float32)           # [3, N]
    a1[0:3] = t1
    a1[3:6] = t1 * t1
    a1[6:9] = 1.0
    a2 = np.empty((9, M), dtype=np.float32)
    t2 = xyz2_b.T.astype(np.float32)           # [3, M]
    # negated so the PE emits -d2: all on-chip folds become MAX
    # (GPSIMD only implements tensor_max, not min)
    a2[0:3] = 2.0 * t2
    a2[3:6] = -1.0
    a2[6:9] = -(t2 * t2)
    return a1, a2


def run_cores(xyz1, xyz2, **kw):
    """Run the per-core kernel on all 8 cores; returns BassKernelResults."""
    xyz1 = np.asarray(xyz1, dtype=np.float32)
    xyz2 = np.asarray(xyz2, dtype=np.float32)
    assert xyz1.shape == (B, N, D) and xyz2.shape == (B, M, D)
    in_maps = []
    for b in range(B):
        a1, a2 = _augment(xyz1[b], xyz2[b])
        in_maps.append({"aug1": a1, "aug2": a2})
    return run_bass_kernel_spmd(_get_nc(), in_maps, list(range(B)), **kw)


def _combine(results):
    parts = np.stack([r["partial"][0] for r in results])  # [8, 2]
    s1 = float(parts[:, 0].astype(np.float64).sum())
    s2 = float(parts[:, 1].astype(np.float64).sum())
    return np.asarray(-(s1 / (B * N) + s2 / (B * M)), dtype=np.float32)


def kernel(xyz1, xyz2):
    res = run_cores(xyz1, xyz2)
    return _combine(res.results)



# revision 43
# speedup vs baseline: 1.1708x; 1.1708x over previous
"""Chamfer distance kernel for Trainium2 (Bass/Tile), 8 NeuronCores.

Full inputs: xyz1 [8, 4096, 3] f32, xyz2 [8, 4096, 3] f32.
Output: scalar f32 = mean(min_m d2[b,n,m]) + mean(min_n d2[b,n,m]).

Sharding: data-parallel over batch B=8, one batch element per core.
Each core computes partial sums [sum(dist1), sum(dist2)]; host combines
(and negates: the device works on -d2 throughout).

Per-core algorithm: the NEGATED d2 matrix is produced by a K=9 f32r
matmul over host-side augmented operands:
  aug1 rows = [x1_d (3), x1_d^2 (3), 1 (3)]      (lhsT, [9, 4096])
  aug2 rows = [2*x2_d (3), -1 (3), -x2_d^2 (3)]  (rhs,  [9, 4096])
  psum[n, m] = -d2[n, m]
All folds are MAX (min d2 == max -d2).

Work distribution per [128 x 4096] chunk-plane (cost-model rates:
ACT 0.83 ns/elem, DVE 1x 1.04 / 2x 0.52 ns/elem, GPSIMD 1.39 ns/elem;
DVE may read at most one PSUM operand per op; GPSIMD is SBUF-only and
has no elementwise ops -- only partition_all_reduce; the fused
tensor_tensor_reduce instruction crashes the NEFF runtime, so dist1
uses a fold pyramid):
  - PE (~38% busy): 8 matmuls -> two [128 x 2048] PSUM tiles.
  - ACT (critical engine, ~122us): downcasts both tiles into a bf16
    staging plane.
  - DVE dist1: per-plane first fold (bf16 2x), then the upper pyramid
    levels batched 4 planes per instruction to amortize op overheads.
    D1 slots are filled in arrival order (their sum is order-blind).
  - dist2 splits across DVE and GPSIMD:
      * GPS_PLANES (14 early even chunks): gpsimd partition_all_reduce
        folds the plane's 128 rows -> a replicated column-max row; a
        row-DMA drops it into one partition of a collector tile; one
        more partition_all_reduce merges the collector into `prs` well
        before the epilogue.
      * the rest (18 planes): DVE running max into R (bf16 2x mode).
        A chain, not a tree: chain latency hides behind ACT's staging
        stream and the post-stream cascade is a single fold.
Epilogue (quarter-pipelined to shorten the tail): the last two chunks'
R-folds run per quarter; each quarter immediately flows through the
GPSIMD partition fold, the [1, M]-level merge with `prs`, and an ACT
accumulator row-sum; the last chunk's dist1 pyramid fills the DVE wait
gaps.  dist1's partition-sum uses a tiny PE ones-matmul.
"""

import numpy as np
from contextlib import ExitStack

import concourse.bass as bass
import concourse.bacc as bacc
import concourse.bass_isa as bass_isa
import concourse.mybir as mybir
from concourse.tile import TileContext
from concourse.bass_utils import run_bass_kernel_spmd

B, N, M, D = 8, 4096, 4096, 3
P = 128            # partitions (n-chunk size)
NI = N // P        # 32 n-chunks
FREE = 2048        # columns per PSUM tile (4 banks)
MM = 512           # matmul free dim (1 PSUM bank)
FDT = mybir.dt.float32
FRT = mybir.dt.float32r   # same bits as f32; PE streams 1 row/cycle (vs 4)
BDT = mybir.dt.bfloat16
AX = mybir.AxisListType
MAX = mybir.AluOpType.max
ADD = mybir.AluOpType.add

# ---- tuning knobs ----
# chunks whose dist2 plane-fold runs on GPSIMD: early even chunks, so
# arrivals (7.6us apart) outpace the 5.7us fold and the collector merge
# lands before the epilogue needs it
GPS_PLANES = tuple(range(0, 27, 2))   # 14 planes
# ramp chunk built from narrower PSUM tiles: each tile = its own
# matmuls + its own ACT copy, so the first copy starts ~3us earlier
PSUM_SLICE = {0: 1024, 1: 1024}
ST_BUFS = 7        # staging lookahead (8KB/partition each)
BG = 4             # dist1 pyramid batch (planes per upper-level op)

_CACHE = {}


def _build():
    nc = bacc.Bacc(None, target_bir_lowering=False)
    a1 = nc.dram_tensor("aug1", [9, N], FRT, kind="ExternalInput")
    a2 = nc.dram_tensor("aug2", [9, M], FRT, kind="ExternalInput")
    part = nc.dram_tensor("partial", [1, 2], FDT, kind="ExternalOutput")

    gps_planes = set(GPS_PLANES)
    n_gps = len(gps_planes)
    dve_planes = [i for i in range(NI) if i not in gps_planes]
    last = NI - 1
    q = M // 4

    with ExitStack() as ctx:
        tc = ctx.enter_context(TileContext(nc))
        sb = ctx.enter_context(tc.tile_pool(name="sb", bufs=1))
        stg = ctx.enter_context(tc.tile_pool(name="stg", bufs=ST_BUFS))
        stf = ctx.enter_context(tc.tile_pool(name="stf", bufs=2))
        stu = ctx.enter_context(tc.tile_pool(name="stu", bufs=1))
        prp = ctx.enter_context(tc.tile_pool(name="prp", bufs=2))
        pp = ctx.enter_context(tc.tile_pool(name="pp", bufs=2, space="PSUM"))

        # PE p-state warmup: a trivial matmul right at t=0 starts the ramp
        # clock so chunk 0's real matmuls run at mid/full speed; the ACT
        # copy pulls the activation-table load into the DMA window
        w0 = sb.tile([1, 1], FDT)
        w1 = sb.tile([1, 1], FDT)
        nc.vector.memset(w0[:, :], 0.0)
        pwt = pp.tile([1, 1], FDT, tag="pt")
        nc.tensor.matmul(pwt[:, :], w0[:, :], w0[:, :], start=True, stop=True)
        nc.scalar.copy(w1[:, :], w0[:, :])

        # split input DMAs so the first matmuls / first chunk start early
        aug1 = sb.tile([9, N], FRT)
        aug2 = sb.tile([9, M], FRT)
        nc.sync.dma_start(out=aug1[:, 0:P], in_=a1[:, 0:P])
        for k in range(4):
            nc.sync.dma_start(
                out=aug2[:, k * MM:(k + 1) * MM], in_=a2[:, k * MM:(k + 1) * MM],
            )
        nc.sync.dma_start(out=aug2[:, FREE:M], in_=a2[:, FREE:M])
        nc.sync.dma_start(out=aug1[:, P:N], in_=a1[:, P:N])

        D1 = sb.tile([P, NI], FDT)         # full-row max per plane (arrival order)
        coll = sb.tile([n_gps, M], BDT)    # per-GPS-plane column maxes
        R = sb.tile([P, M], BDT)           # DVE-side running max
        prs = sb.tile([P, M], BDT)         # merged collector (replicated)
        Rr = sb.tile([P, M], BDT)          # partition-folded R
        Rm = sb.tile([1, M], BDT)
        s2h = sb.tile([1, 4], FDT)
        out_t = sb.tile([1, 2], FDT)

        # dist1 pyramid state: per-plane L1 results accumulate into a
        # [P, BG, FREE] batch tile; upper levels run once per full batch
        batch = {"tile": None, "n": 0, "base": 0}

        def d1_l1(st):
            """First dist1 fold for a staged plane (bf16 2x)."""
            if batch["tile"] is None:
                bl1 = stf.tile([P, BG, FREE], BDT, tag="bl1")
                batch["tile"] = bl1
            bl = batch["tile"]
            nc.vector.tensor_tensor(
                out=bl[:, batch["n"], :], in0=st[:, 0:FREE], in1=st[:, FREE:M],
                op=MAX,
            )
            batch["n"] += 1

        def d1_upper():
            """Batched upper pyramid: [P, BG, 2048] -> D1 arrival slots."""
            bl, base = batch["tile"], batch["base"]
            assert batch["n"] == BG
            w = FREE
            prev = bl[:, :, :]
            for lvl in range(3):
                nxt = stu.tile([P, BG, w // 2], BDT, tag=f"u{lvl}")
                nc.vector.tensor_tensor(
                    out=nxt[:, :, :], in0=prev[:, :, 0:w // 2],
                    in1=prev[:, :, w // 2:w], op=MAX,
                )
                prev, w = nxt[:, :, :], w // 2
            nc.vector.tensor_reduce(
                out=D1[:, base:base + BG], in_=prev, axis=AX.X, op=MAX,
            )
            batch.update(tile=None, n=0, base=base + BG)

        gslot = 0
        first_dve = dve_planes[0]
        defer = {NI - 1}               # last plane: folded in the tail
        defer_st = {}
        for i in range(NI):
            # ---- PE: chunk i -> PSUM tiles; ACT: stage to bf16 ----
            st = stg.tile([P, M], BDT, tag="st")
            pw = PSUM_SLICE.get(i, FREE)
            for c0 in range(0, M, pw):
                pt = pp.tile([P, pw], FDT, tag="pt")
                for k in range(pw // MM):
                    nc.tensor.matmul(
                        pt[:, k * MM:(k + 1) * MM],
                        aug1[:, i * P:(i + 1) * P],
                        aug2[:, c0 + k * MM: c0 + (k + 1) * MM],
                        start=True, stop=True,
                    )
                nc.scalar.copy(st[:, c0:c0 + pw], pt[:, :])
            # ---- DVE dist1 (the last chunk's moves into the tail) ----
            if i != last:
                d1_l1(st)
                if batch["n"] == BG:
                    d1_upper()
            # ---- dist2 ----
            if i in gps_planes:
                pr = prp.tile([P, M], BDT, tag="pr")
                nc.gpsimd.partition_all_reduce(
                    pr[:, :], st[:, :], P, bass_isa.ReduceOp.max,
                )
                # result is replicated across partitions; stash row 0 into
                # this plane's collector slot (SP-queue DMA, off-engine)
                nc.sync.dma_start(
                    out=coll[gslot:gslot + 1, :], in_=pr[0:1, :],
                )
                gslot += 1
                if gslot == n_gps:
                    # merge the collector as soon as the last GPS plane
                    # lands -- well before the epilogue reads `prs`
                    nc.gpsimd.partition_all_reduce(
                        prs[0:n_gps, :], coll[:, :], n_gps,
                        bass_isa.ReduceOp.max,
                    )
            elif i == first_dve:
                nc.vector.tensor_copy(out=R[:, :], in_=st[:, :])
            elif i in defer:
                defer_st[i] = st
            else:
                nc.vector.tensor_tensor(
                    out=R[:, :], in0=st[:, :], in1=R[:, :], op=MAX,
                )

        # ---- tail: quarter-pipelined dist2 epilogue ----
        # Fold the deferred last planes per quarter so each quarter flows
        # through the GPSIMD partition fold as soon as it is final.  DVE
        # issue order interleaves the quarter folds with the collector
        # merges (DVE is in-order), and the last chunk's dist1 pyramid
        # fills the GPSIMD-wait gaps; row-sums ride ACT's accumulator.
        jrow = sb.tile([1, q], BDT)

        def quarter_fold(qq):
            sl = slice(qq * q, (qq + 1) * q)
            for i in sorted(defer_st):
                nc.vector.tensor_tensor(
                    out=R[:, sl], in0=defer_st[i][:, sl], in1=R[:, sl],
                    op=MAX,
                )
            nc.gpsimd.partition_all_reduce(
                Rr[:, sl], R[:, sl], P, bass_isa.ReduceOp.max,
            )

        def quarter_merge(qq):
            sl = slice(qq * q, (qq + 1) * q)
            nc.vector.tensor_tensor(
                out=Rm[0:1, sl], in0=Rr[0:1, sl], in1=prs[0:1, sl], op=MAX,
            )
            nc.scalar.activation(
                out=jrow[0:1, :], in_=Rm[0:1, sl],
                func=mybir.ActivationFunctionType.Copy,
                accum_out=s2h[0:1, qq:qq + 1],
            )

        quarter_fold(0)
        quarter_fold(1)
        d1_l1(defer_st[last])
        quarter_merge(0)
        quarter_fold(2)
        d1_upper()
        quarter_merge(1)
        quarter_fold(3)
        quarter_merge(2)
        quarter_merge(3)

        nc.vector.tensor_reduce(
            out=out_t[0:1, 1:2], in_=s2h[0:1, :], axis=AX.X, op=ADD,
        )

        # ---- dist1 epilogue: sum over planes, partition-sum via PE ----
        s1 = sb.tile([P, 1], FDT)
        nc.vector.tensor_reduce(out=s1[:, :], in_=D1[:, :], axis=AX.X, op=ADD)
        ones_col = sb.tile([P, 1], FDT)
        nc.vector.memset(ones_col[:, :], 1.0)
        p1 = pp.tile([1, 1], FDT, tag="pt")
        nc.tensor.matmul(p1[:, :], s1[:, :], ones_col[:, :], start=True, stop=True)
        nc.vector.tensor_copy(out=out_t[0:1, 0:1], in_=p1[0:1, 0:1])

        nc.sync.dma_start(out=part[:, :], in_=out_t[0:1, :])

    nc.compile()
    return nc


def _get_nc():
    if "nc" not in _CACHE:
        _CACHE["nc"] = _build()
    return _CACHE["nc"]


def _augment(xyz1_b, xyz2_b):
    """Host-side layout of one batch element into the augmented operands."""
    a1 = np.empty((9, N), dtype=np.float32)
    t1 = xyz1_b.T.astype(np.float32)           # [3, N]
    a1[0:3] = t1
    a1[3:6] = t1 * t1
    a1[6:9] = 1.0
    a2 = np.empty((9, M), dtype=np.float32)
    t2 = xyz2_b.T.astype(np.float32)           # [3, M]
    # negated so the PE emits -d2: all on-chip folds become MAX
    # (GPSIMD's partition_all_reduce implements max, not min)
    a2[0:3] = 2.0 * t2
    a2[3:6] = -1.0
    a2[6:9] = -(t2 * t2)
    return a1, a2


def run_cores(xyz1, xyz2, **kw):
    """Run the per-core kernel on all 8 cores; returns BassKernelResults."""
    xyz1 = np.asarray(xyz1, dtype=np.float32)
    xyz2 = np.asarray(xyz2, dtype=np.float32)
    assert xyz1.shape == (B, N, D) and xyz2.shape == (B, M, D)
    in_maps = []
    for b in range(B):
        a1, a2 = _augment(xyz1[b], xyz2[b])
        in_maps.append({"aug1": a1, "aug2": a2})
    return run_bass_kernel_spmd(_get_nc(), in_maps, list(range(B)), **kw)


def _combine(results):
    parts = np.stack([r["partial"][0] for r in results])  # [8, 2]
    s1 = float(parts[:, 0].astype(np.float64).sum())
    s2 = float(parts[:, 1].astype(np.float64).sum())
    return np.asarray(-(s1 / (B * N) + s2 / (B * M)), dtype=np.float32)


def kernel(xyz1, xyz2):
    res = run_cores(xyz1, xyz2)
    return _combine(res.results)


# revision 48
# speedup vs baseline: 1.1770x; 1.0053x over previous
"""Chamfer distance kernel for Trainium2 (Bass/Tile), 8 NeuronCores.

Full inputs: xyz1 [8, 4096, 3] f32, xyz2 [8, 4096, 3] f32.
Output: scalar f32 = mean(min_m d2[b,n,m]) + mean(min_n d2[b,n,m]).

Sharding: data-parallel over batch B=8, one batch element per core.
Each core computes partial sums [sum(dist1), sum(dist2)]; host combines
(and negates: the device works on -d2 throughout).

Per-core algorithm: the NEGATED d2 matrix is produced by a K=9 f32r
matmul over host-side augmented operands:
  aug1 rows = [x1_d (3), x1_d^2 (3), 1 (3)]      (lhsT, [9, 4096])
  aug2 rows = [2*x2_d (3), -1 (3), -x2_d^2 (3)]  (rhs,  [9, 4096])
  psum[n, m] = -d2[n, m]
All folds are MAX (min d2 == max -d2).

Work distribution per [128 x 4096] chunk-plane (cost-model rates:
ACT 0.83 ns/elem, DVE 1x 1.04 / 2x 0.52 ns/elem, GPSIMD 1.39 ns/elem;
DVE may read at most one PSUM operand per op; GPSIMD is SBUF-only and
has no elementwise ops -- only partition_all_reduce; the fused
tensor_tensor_reduce instruction crashes the NEFF runtime, so dist1
uses a fold pyramid):
  - PE (~38% busy): 8 matmuls -> two [128 x 2048] PSUM tiles.
  - ACT (critical engine, ~122us): downcasts both tiles into a bf16
    staging plane.
  - DVE dist1: per-plane first fold (bf16 2x), then the upper pyramid
    levels batched 4 planes per instruction to amortize op overheads.
    D1 slots are filled in arrival order (their sum is order-blind).
  - dist2 splits across DVE and GPSIMD:
      * GPS_PLANES (14 early even chunks): gpsimd partition_all_reduce
        folds the plane's 128 rows -> a replicated column-max row; a
        row-DMA drops it into one partition of a collector tile; one
        more partition_all_reduce merges the collector into `prs` well
        before the epilogue.
      * the rest (18 planes): DVE running max into R (bf16 2x mode).
        A chain, not a tree: chain latency hides behind ACT's staging
        stream and the post-stream cascade is a single fold.
Epilogue (quarter-pipelined to shorten the tail): the last chunk is
staged into two half tiles (so its folds start after the first half
lands) and its R-folds run per quarter; each quarter immediately flows
through the GPSIMD partition fold, the [1, M]-level merge with `prs`,
and an ACT-accumulator row-sum; the last chunk's dist1 fold fills the
DVE wait gaps.  dist1's partition-sum uses a tiny PE ones-matmul.

Timeline (cost model): ~145.5us/core wall; ACT ~127.6us busy (the 64
PSUM->SBUF downcasts ARE the critical resource), DVE ~117.5, GPSIMD
~93.3, PE ~54.5.  Baseline being improved on: 171.3us (DVE-bound).
"""

import numpy as np
from contextlib import ExitStack

import concourse.bass as bass
import concourse.bacc as bacc
import concourse.bass_isa as bass_isa
import concourse.mybir as mybir
from concourse.tile import TileContext
from concourse.bass_utils import run_bass_kernel_spmd

B, N, M, D = 8, 4096, 4096, 3
P = 128            # partitions (n-chunk size)
NI = N // P        # 32 n-chunks
FREE = 2048        # columns per PSUM tile (4 banks)
MM = 512           # matmul free dim (1 PSUM bank)
FDT = mybir.dt.float32
FRT = mybir.dt.float32r   # same bits as f32; PE streams 1 row/cycle (vs 4)
BDT = mybir.dt.bfloat16
AX = mybir.AxisListType
MAX = mybir.AluOpType.max
ADD = mybir.AluOpType.add

# ---- tuning knobs ----
# chunks whose dist2 plane-fold runs on GPSIMD: early even chunks, so
# arrivals (7.6us apart) outpace the 5.7us fold and the collector merge
# lands before the epilogue needs it
GPS_PLANES = tuple(range(0, 27, 2))   # 14 planes
# ramp chunk built from narrower PSUM tiles: each tile = its own
# matmuls + its own ACT copy, so the first copy starts ~3us earlier
PSUM_SLICE = {0: 1024, 1: 1024}
ST_BUFS = 7        # staging lookahead (8KB/partition each)
BG = 4             # dist1 pyramid batch (planes per upper-level op)

_CACHE = {}


def _build():
    nc = bacc.Bacc(None, target_bir_lowering=False)
    a1 = nc.dram_tensor("aug1", [9, N], FRT, kind="ExternalInput")
    a2 = nc.dram_tensor("aug2", [9, M], FRT, kind="ExternalInput")
    part = nc.dram_tensor("partial", [1, 2], FDT, kind="ExternalOutput")

    gps_planes = set(GPS_PLANES)
    n_gps = len(gps_planes)
    dve_planes = [i for i in range(NI) if i not in gps_planes]
    last = NI - 1
    q = M // 4

    with ExitStack() as ctx:
        tc = ctx.enter_context(TileContext(nc))
        sb = ctx.enter_context(tc.tile_pool(name="sb", bufs=1))
        stg = ctx.enter_context(tc.tile_pool(name="stg", bufs=ST_BUFS))
        stf = ctx.enter_context(tc.tile_pool(name="stf", bufs=2))
        stu = ctx.enter_context(tc.tile_pool(name="stu", bufs=1))
        prp = ctx.enter_context(tc.tile_pool(name="prp", bufs=2))
        pp = ctx.enter_context(tc.tile_pool(name="pp", bufs=2, space="PSUM"))

        # PE p-state warmup: a trivial matmul right at t=0 starts the ramp
        # clock so chunk 0's real matmuls run at mid/full speed; the ACT
        # copy pulls the activation-table load into the DMA window
        w0 = sb.tile([1, 1], FDT)
        w1 = sb.tile([1, 1], FDT)
        nc.vector.memset(w0[:, :], 0.0)
        pwt = pp.tile([1, 1], FDT, tag="pt")
        nc.tensor.matmul(pwt[:, :], w0[:, :], w0[:, :], start=True, stop=True)
        nc.scalar.copy(w1[:, :], w0[:, :])

        # split input DMAs so the first matmuls / first chunk start early
        aug1 = sb.tile([9, N], FRT)
        aug2 = sb.tile([9, M], FRT)
        nc.sync.dma_start(out=aug1[:, 0:P], in_=a1[:, 0:P])
        for k in range(4):
            nc.sync.dma_start(
                out=aug2[:, k * MM:(k + 1) * MM], in_=a2[:, k * MM:(k + 1) * MM],
            )
        nc.sync.dma_start(out=aug2[:, FREE:M], in_=a2[:, FREE:M])
        nc.sync.dma_start(out=aug1[:, P:N], in_=a1[:, P:N])

        D1 = sb.tile([P, NI], FDT)         # full-row max per plane (arrival order)
        coll = sb.tile([n_gps, M], BDT)    # per-GPS-plane column maxes
        R = sb.tile([P, M], BDT)           # DVE-side running max
        prs = sb.tile([P, M], BDT)         # merged collector (replicated)
        Rr = sb.tile([P, M], BDT)          # partition-folded R
        Rm = sb.tile([1, M], BDT)
        s2h = sb.tile([1, 4], FDT)
        out_t = sb.tile([1, 2], FDT)

        # dist1 pyramid state: per-plane L1 results accumulate into a
        # [P, BG, FREE] batch tile; upper levels run once per full batch
        batch = {"tile": None, "n": 0, "base": 0}

        def d1_l1(st):
            """First dist1 fold for a staged plane (bf16 2x)."""
            if batch["tile"] is None:
                bl1 = stf.tile([P, BG, FREE], BDT, tag="bl1")
                batch["tile"] = bl1
            bl = batch["tile"]
            nc.vector.tensor_tensor(
                out=bl[:, batch["n"], :], in0=st[:, 0:FREE], in1=st[:, FREE:M],
                op=MAX,
            )
            batch["n"] += 1

        def d1_upper():
            """Batched upper pyramid: [P, BG, 2048] -> D1 arrival slots."""
            bl, base = batch["tile"], batch["base"]
            assert batch["n"] == BG
            w = FREE
            prev = bl[:, :, :]
            for lvl in range(3):
                nxt = stu.tile([P, BG, w // 2], BDT, tag=f"u{lvl}")
                nc.vector.tensor_tensor(
                    out=nxt[:, :, :], in0=prev[:, :, 0:w // 2],
                    in1=prev[:, :, w // 2:w], op=MAX,
                )
                prev, w = nxt[:, :, :], w // 2
            nc.vector.tensor_reduce(
                out=D1[:, base:base + BG], in_=prev, axis=AX.X, op=MAX,
            )
            batch.update(tile=None, n=0, base=base + BG)

        gslot = 0
        first_dve = dve_planes[0]
        defer = {NI - 1}               # last plane: folded in the tail
        defer_st = {}
        last_halves = None
        for i in range(NI):
            # ---- PE: chunk i -> PSUM tiles; ACT: stage to bf16 ----
            # The last chunk stages into two half tiles: tile deps are
            # writer-granular, so the tail's first quarter folds can start
            # as soon as the low half lands (~2us earlier)
            if i == last:
                sta = stg.tile([P, FREE], BDT, tag="sta", bufs=1)
                stb = stg.tile([P, FREE], BDT, tag="stb", bufs=1)
                last_halves = (sta, stb)
                halves = [sta[:, :], stb[:, :]]
            else:
                st = stg.tile([P, M], BDT, tag="st")
                halves = [st[:, 0:FREE], st[:, FREE:M]]
            pw = PSUM_SLICE.get(i, FREE)
            for c0 in range(0, M, pw):
                pt = pp.tile([P, pw], FDT, tag="pt")
                for k in range(pw // MM):
                    nc.tensor.matmul(
                        pt[:, k * MM:(k + 1) * MM],
                        aug1[:, i * P:(i + 1) * P],
                        aug2[:, c0 + k * MM: c0 + (k + 1) * MM],
                        start=True, stop=True,
                    )
                h, hoff = divmod(c0, FREE)
                nc.scalar.copy(halves[h][:, hoff:hoff + pw], pt[:, :])
            # ---- DVE dist1 (the last chunk's moves into the tail) ----
            if i != last:
                d1_l1(st)
                if batch["n"] == BG:
                    d1_upper()
            # ---- dist2 ----
            if i in gps_planes:
                pr = prp.tile([P, M], BDT, tag="pr")
                nc.gpsimd.partition_all_reduce(
                    pr[:, :], st[:, :], P, bass_isa.ReduceOp.max,
                )
                # result is replicated across partitions; stash row 0 into
                # this plane's collector slot (SP-queue DMA, off-engine)
                nc.sync.dma_start(
                    out=coll[gslot:gslot + 1, :], in_=pr[0:1, :],
                )
                gslot += 1
                if gslot == n_gps:
                    # merge the collector as soon as the last GPS plane
                    # lands -- well before the epilogue reads `prs`
                    nc.gpsimd.partition_all_reduce(
                        prs[0:n_gps, :], coll[:, :], n_gps,
                        bass_isa.ReduceOp.max,
                    )
            elif i == first_dve:
                nc.vector.tensor_copy(out=R[:, :], in_=st[:, :])
            elif i in defer:
                pass  # folded in the tail via last_halves
            else:
                nc.vector.tensor_tensor(
                    out=R[:, :], in0=st[:, :], in1=R[:, :], op=MAX,
                )

        # ---- tail: quarter-pipelined dist2 epilogue ----
        # Fold the deferred last planes per quarter so each quarter flows
        # through the GPSIMD partition fold as soon as it is final.  DVE
        # issue order interleaves the quarter folds with the collector
        # merges (DVE is in-order), and the last chunk's dist1 pyramid
        # fills the GPSIMD-wait gaps; row-sums ride ACT's accumulator.
        jrow = sb.tile([1, q], BDT)

        def quarter_fold(qq):
            sl = slice(qq * q, (qq + 1) * q)
            half = last_halves[qq // 2]
            hsl = slice((qq % 2) * q, (qq % 2 + 1) * q)
            nc.vector.tensor_tensor(
                out=R[:, sl], in0=half[:, hsl], in1=R[:, sl], op=MAX,
            )
            nc.gpsimd.partition_all_reduce(
                Rr[:, sl], R[:, sl], P, bass_isa.ReduceOp.max,
            )

        def quarter_merge(qq):
            sl = slice(qq * q, (qq + 1) * q)
            nc.vector.tensor_tensor(
                out=Rm[0:1, sl], in0=Rr[0:1, sl], in1=prs[0:1, sl], op=MAX,
            )
            nc.scalar.activation(
                out=jrow[0:1, :], in_=Rm[0:1, sl],
                func=mybir.ActivationFunctionType.Copy,
                accum_out=s2h[0:1, qq:qq + 1],
            )

        quarter_fold(0)
        quarter_fold(1)
        # last chunk's dist1 first fold: the two half tiles directly
        bl = batch["tile"]
        nc.vector.tensor_tensor(
            out=bl[:, batch["n"], :], in0=last_halves[0][:, :],
            in1=last_halves[1][:, :], op=MAX,
        )
        batch["n"] += 1
        quarter_merge(0)
        quarter_fold(2)
        d1_upper()
        quarter_merge(1)
        quarter_fold(3)
        quarter_merge(2)
        quarter_merge(3)

        nc.vector.tensor_reduce(
            out=out_t[0:1, 1:2], in_=s2h[0:1, :], axis=AX.X, op=ADD,
        )

        # ---- dist1 epilogue: sum over planes, partition-sum via PE ----
        s1 = sb.tile([P, 1], FDT)
        nc.vector.tensor_reduce(out=s1[:, :], in_=D1[:, :], axis=AX.X, op=ADD)
        ones_col = sb.tile([P, 1], FDT)
        nc.vector.memset(ones_col[:, :], 1.0)
        p1 = pp.tile([1, 1], FDT, tag="pt")
        nc.tensor.matmul(p1[:, :], s1[:, :], ones_col[:, :], start=True, stop=True)
        nc.vector.tensor_copy(out=out_t[0:1, 0:1], in_=p1[0:1, 0:1])

        nc.sync.dma_start(out=part[:, :], in_=out_t[0:1, :])

    nc.compile()
    return nc


def _get_nc():
    if "nc" not in _CACHE:
        _CACHE["nc"] = _build()
    return _CACHE["nc"]


def _augment(xyz1_b, xyz2_b):
    """Host-side layout of one batch element into the augmented operands."""
    a1 = np.empty((9, N), dtype=np.float32)
    t1 = xyz1_b.T.astype(np.float32)           # [3, N]
    a1[0:3] = t1
    a1[3:6] = t1 * t1
    a1[6:9] = 1.0
    a2 = np.empty((9, M), dtype=np.float32)
    t2 = xyz2_b.T.astype(np.float32)           # [3, M]
    # negated so the PE emits -d2: all on-chip folds become MAX
    # (GPSIMD's partition_all_reduce implements max, not min)
    a2[0:3] = 2.0 * t2
    a2[3:6] = -1.0
    a2[6:9] = -(t2 * t2)
    return a1, a2


def run_cores(xyz1, xyz2, **kw):
    """Run the per-core kernel on all 8 cores; returns BassKernelResults."""
    xyz1 = np.asarray(xyz1, dtype=np.float32)
    xyz2 = np.asarray(xyz2, dtype=np.float32)
    assert xyz1.shape == (B, N, D) and xyz2.shape == (B, M, D)
    in_maps = []
    for b in range(B):
        a1, a2 = _augment(xyz1[b], xyz2[b])
        in_maps.append({"aug1": a1, "aug2": a2})
    return run_bass_kernel_spmd(_get_nc(), in_maps, list(range(B)), **kw)


def _combine(results):
    parts = np.stack([r["partial"][0] for r in results])  # [8, 2]
    s1 = float(parts[:, 0].astype(np.float64).sum())
    s2 = float(parts[:, 1].astype(np.float64).sum())
    return np.asarray(-(s1 / (B * N) + s2 / (B * M)), dtype=np.float32)


def kernel(xyz1, xyz2):
    res = run_cores(xyz1, xyz2)
    return _combine(res.results)


# revision 51
# speedup vs baseline: 1.1955x; 1.0157x over previous
"""Chamfer distance kernel for Trainium2 (Bass/Tile), 8 NeuronCores.

Full inputs: xyz1 [8, 4096, 3] f32, xyz2 [8, 4096, 3] f32.
Output: scalar f32 = mean(min_m d2[b,n,m]) + mean(min_n d2[b,n,m]).

Sharding: data-parallel over batch B=8, one batch element per core.
Each core computes partial sums [sum(dist1), sum(dist2)]; host combines
(and negates: the device works on -d2 throughout).

Per-core algorithm: the NEGATED d2 matrix is produced by a K=9 f32r
matmul over host-side augmented operands:
  aug1 rows = [x1_d (3), x1_d^2 (3), 1 (3)]      (lhsT, [9, 4096])
  aug2 rows = [2*x2_d (3), -1 (3), -x2_d^2 (3)]  (rhs,  [9, 4096])
  psum[n, m] = -d2[n, m]
All folds are MAX (min d2 == max -d2).

Work distribution per [128 x 4096] chunk-plane (cost-model rates:
ACT 0.83 ns/elem, DVE 1x 1.04 / 2x 0.52 ns/elem, GPSIMD 1.39 ns/elem;
DVE may read at most one PSUM operand per op; GPSIMD is SBUF-only and
has no elementwise ops -- only partition_all_reduce; the fused
tensor_tensor_reduce instruction crashes the NEFF runtime, so dist1
uses a fold pyramid):
  - PE (~38% busy): 8 matmuls -> two [128 x 2048] PSUM tiles.
  - ACT (critical engine, ~122us): downcasts both tiles into a bf16
    staging plane.
  - DVE dist1: per-plane first fold (bf16 2x), then the upper pyramid
    levels batched 4 planes per instruction to amortize op overheads.
    D1 slots are filled in arrival order (their sum is order-blind).
  - dist2 splits across DVE and GPSIMD:
      * GPS_PLANES (14 early even chunks): gpsimd partition_all_reduce
        folds the plane's 128 rows -> a replicated column-max row; a
        row-DMA drops it into one partition of a collector tile; one
        more partition_all_reduce merges the collector into `prs` well
        before the epilogue.
      * the rest (18 planes): DVE running max into R (bf16 2x mode).
        A chain, not a tree: chain latency hides behind ACT's staging
        stream and the post-stream cascade is a single fold.
Epilogue (quarter-pipelined to shorten the tail): the last chunk is
staged into two half tiles (so its folds start after the first half
lands) and its R-folds run per quarter; each quarter immediately flows
through the GPSIMD partition fold, the [1, M]-level merge with `prs`,
and an ACT-accumulator row-sum; the last chunk's dist1 fold fills the
DVE wait gaps.  dist1's partition-sum uses a tiny PE ones-matmul.

Timeline (cost model): ~145.5us/core wall; ACT ~127.6us busy (the 64
PSUM->SBUF downcasts ARE the critical resource), DVE ~117.5, GPSIMD
~93.3, PE ~54.5.  Baseline being improved on: 171.3us (DVE-bound).
"""

import numpy as np
from contextlib import ExitStack

import concourse.bass as bass
import concourse.bacc as bacc
import concourse.bass_isa as bass_isa
import concourse.mybir as mybir
from concourse.tile import TileContext
from concourse.bass_utils import run_bass_kernel_spmd

B, N, M, D = 8, 4096, 4096, 3
P = 128            # partitions (n-chunk size)
NI = N // P        # 32 n-chunks
FREE = 2048        # columns per PSUM tile (4 banks)
MM = 512           # matmul free dim (1 PSUM bank)
FDT = mybir.dt.float32
FRT = mybir.dt.float32r   # same bits as f32; PE streams 1 row/cycle (vs 4)
BDT = mybir.dt.bfloat16
AX = mybir.AxisListType
MAX = mybir.AluOpType.max
ADD = mybir.AluOpType.add

# ---- tuning knobs ----
# chunks whose dist2 plane-fold runs on GPSIMD: early even chunks, so
# arrivals (7.6us apart) outpace the 5.7us fold and the collector merge
# lands before the epilogue needs it
GPS_PLANES = tuple(range(0, 29, 2))   # 15 planes
# ramp chunk built from narrower PSUM tiles: each tile = its own
# matmuls + its own ACT copy, so the first copy starts ~3us earlier
PSUM_SLICE = {0: 1024, 1: 1024}
ST_BUFS = 8        # staging lookahead (8KB/partition each)
BG = 4             # dist1 pyramid batch (planes per upper-level op)

_CACHE = {}


def _build():
    nc = bacc.Bacc(None, target_bir_lowering=False)
    a1 = nc.dram_tensor("aug1", [9, N], FRT, kind="ExternalInput")
    a2 = nc.dram_tensor("aug2", [9, M], FRT, kind="ExternalInput")
    part = nc.dram_tensor("partial", [1, 2], FDT, kind="ExternalOutput")

    gps_planes = set(GPS_PLANES)
    n_gps = len(gps_planes)
    dve_planes = [i for i in range(NI) if i not in gps_planes]
    last = NI - 1
    q = M // 4

    with ExitStack() as ctx:
        tc = ctx.enter_context(TileContext(nc))
        sb = ctx.enter_context(tc.tile_pool(name="sb", bufs=1))
        stg = ctx.enter_context(tc.tile_pool(name="stg", bufs=ST_BUFS))
        stf = ctx.enter_context(tc.tile_pool(name="stf", bufs=2))
        stu = ctx.enter_context(tc.tile_pool(name="stu", bufs=1))
        prp = ctx.enter_context(tc.tile_pool(name="prp", bufs=2))
        pp = ctx.enter_context(tc.tile_pool(name="pp", bufs=2, space="PSUM"))

        # PE p-state warmup: a trivial matmul right at t=0 starts the ramp
        # clock so chunk 0's real matmuls run at mid/full speed; the ACT
        # copy pulls the activation-table load into the DMA window
        w0 = sb.tile([1, 1], FDT)
        w1 = sb.tile([1, 1], FDT)
        nc.vector.memset(w0[:, :], 0.0)
        pwt = pp.tile([1, 1], FDT, tag="pt")
        nc.tensor.matmul(pwt[:, :], w0[:, :], w0[:, :], start=True, stop=True)
        nc.scalar.copy(w1[:, :], w0[:, :])

        # split input DMAs so the first matmuls / first chunk start early
        aug1 = sb.tile([9, N], FRT)
        aug2 = sb.tile([9, M], FRT)
        nc.sync.dma_start(out=aug1[:, 0:P], in_=a1[:, 0:P])
        for k in range(4):
            nc.sync.dma_start(
                out=aug2[:, k * MM:(k + 1) * MM], in_=a2[:, k * MM:(k + 1) * MM],
            )
        nc.sync.dma_start(out=aug2[:, FREE:M], in_=a2[:, FREE:M])
        nc.sync.dma_start(out=aug1[:, P:N], in_=a1[:, P:N])

        D1 = sb.tile([P, NI], FDT)         # full-row max per plane (arrival order)
        coll = sb.tile([n_gps, M], BDT)    # per-GPS-plane column maxes
        R = sb.tile([P, M], BDT)           # DVE-side running max
        prs = sb.tile([P, M], BDT)         # merged collector (replicated)
        Rr = sb.tile([P, M], BDT)          # partition-folded R
        Rm = sb.tile([1, M], BDT)
        s2h = sb.tile([1, 4], FDT)
        out_t = sb.tile([1, 2], FDT)

        # dist1 pyramid state: per-plane L1 results accumulate into a
        # [P, BG, FREE] batch tile; upper levels run once per full batch
        batch = {"tile": None, "n": 0, "base": 0}

        def d1_l1(st):
            """First dist1 fold for a staged plane (bf16 2x)."""
            if batch["tile"] is None:
                bl1 = stf.tile([P, BG, FREE], BDT, tag="bl1")
                batch["tile"] = bl1
            bl = batch["tile"]
            nc.vector.tensor_tensor(
                out=bl[:, batch["n"], :], in0=st[:, 0:FREE], in1=st[:, FREE:M],
                op=MAX,
            )
            batch["n"] += 1

        def d1_upper():
            """Batched upper pyramid: [P, BG, 2048] -> D1 arrival slots."""
            bl, base = batch["tile"], batch["base"]
            assert batch["n"] == BG
            w = FREE
            prev = bl[:, :, :]
            for lvl in range(3):
                nxt = stu.tile([P, BG, w // 2], BDT, tag=f"u{lvl}")
                nc.vector.tensor_tensor(
                    out=nxt[:, :, :], in0=prev[:, :, 0:w // 2],
                    in1=prev[:, :, w // 2:w], op=MAX,
                )
                prev, w = nxt[:, :, :], w // 2
            nc.vector.tensor_reduce(
                out=D1[:, base:base + BG], in_=prev, axis=AX.X, op=MAX,
            )
            batch.update(tile=None, n=0, base=base + BG)

        gslot = 0
        first_dve = dve_planes[0]
        defer = {NI - 1}               # last plane: folded in the tail
        defer_st = {}
        last_halves = None
        for i in range(NI):
            # ---- PE: chunk i -> PSUM tiles; ACT: stage to bf16 ----
            # The last chunk stages into two half tiles: tile deps are
            # writer-granular, so the tail's first quarter folds can start
            # as soon as the low half lands (~2us earlier)
            if i == last:
                sta = stg.tile([P, FREE], BDT, tag="sta", bufs=1)
                stb = stg.tile([P, FREE], BDT, tag="stb", bufs=1)
                last_halves = (sta, stb)
                halves = [sta[:, :], stb[:, :]]
            else:
                st = stg.tile([P, M], BDT, tag="st")
                halves = [st[:, 0:FREE], st[:, FREE:M]]
            pw = PSUM_SLICE.get(i, FREE)
            for c0 in range(0, M, pw):
                pt = pp.tile([P, pw], FDT, tag="pt")
                for k in range(pw // MM):
                    nc.tensor.matmul(
                        pt[:, k * MM:(k + 1) * MM],
                        aug1[:, i * P:(i + 1) * P],
                        aug2[:, c0 + k * MM: c0 + (k + 1) * MM],
                        start=True, stop=True,
                    )
                h, hoff = divmod(c0, FREE)
                nc.scalar.copy(halves[h][:, hoff:hoff + pw], pt[:, :])
            # ---- DVE dist1 (the last chunk's moves into the tail) ----
            if i != last:
                d1_l1(st)
                if batch["n"] == BG:
                    d1_upper()
            # ---- dist2 ----
            if i in gps_planes:
                pr = prp.tile([P, M], BDT, tag="pr")
                nc.gpsimd.partition_all_reduce(
                    pr[:, :], st[:, :], P, bass_isa.ReduceOp.max,
                )
                # result is replicated across partitions; stash row 0 into
                # this plane's collector slot (SP-queue DMA, off-engine)
                nc.sync.dma_start(
                    out=coll[gslot:gslot + 1, :], in_=pr[0:1, :],
                )
                gslot += 1
                if gslot == n_gps:
                    # merge the collector as soon as the last GPS plane
                    # lands -- well before the epilogue reads `prs`
                    nc.gpsimd.partition_all_reduce(
                        prs[0:n_gps, :], coll[:, :], n_gps,
                        bass_isa.ReduceOp.max,
                    )
            elif i == first_dve:
                nc.vector.tensor_copy(out=R[:, :], in_=st[:, :])
            elif i in defer:
                pass  # folded in the tail via last_halves
            else:
                nc.vector.tensor_tensor(
                    out=R[:, :], in0=st[:, :], in1=R[:, :], op=MAX,
                )

        # ---- tail: quarter-pipelined dist2 epilogue ----
        # Fold the deferred last planes per quarter so each quarter flows
        # through the GPSIMD partition fold as soon as it is final.  DVE
        # issue order interleaves the quarter folds with the collector
        # merges (DVE is in-order), and the last chunk's dist1 pyramid
        # fills the GPSIMD-wait gaps; row-sums ride ACT's accumulator.
        jrow = sb.tile([1, q], BDT)

        def quarter_fold(qq):
            sl = slice(qq * q, (qq + 1) * q)
            half = last_halves[qq // 2]
            hsl = slice((qq % 2) * q, (qq % 2 + 1) * q)
            nc.vector.tensor_tensor(
                out=R[:, sl], in0=half[:, hsl], in1=R[:, sl], op=MAX,
            )
            nc.gpsimd.partition_all_reduce(
                Rr[:, sl], R[:, sl], P, bass_isa.ReduceOp.max,
            )

        def quarter_merge(qq):
            sl = slice(qq * q, (qq + 1) * q)
            nc.vector.tensor_tensor(
                out=Rm[0:1, sl], in0=Rr[0:1, sl], in1=prs[0:1, sl], op=MAX,
            )
            nc.scalar.activation(
                out=jrow[0:1, :], in_=Rm[0:1, sl],
                func=mybir.ActivationFunctionType.Copy,
                accum_out=s2h[0:1, qq:qq + 1],
            )

        quarter_fold(0)
        quarter_fold(1)
        # last chunk's dist1 first fold: the two half tiles directly
        bl = batch["tile"]
        nc.vector.tensor_tensor(
            out=bl[:, batch["n"], :], in0=last_halves[0][:, :],
            in1=last_halves[1][:, :], op=MAX,
        )
        batch["n"] += 1
        quarter_merge(0)
        quarter_fold(2)
        d1_upper()
        quarter_merge(1)
        quarter_fold(3)
        quarter_merge(2)
        quarter_merge(3)

        nc.vector.tensor_reduce(
            out=out_t[0:1, 1:2], in_=s2h[0:1, :], axis=AX.X, op=ADD,
        )

        # ---- dist1 epilogue: sum over planes, partition-sum via PE ----
        s1 = sb.tile([P, 1], FDT)
        nc.vector.tensor_reduce(out=s1[:, :], in_=D1[:, :], axis=AX.X, op=ADD)
        ones_col = sb.tile([P, 1], FDT)
        nc.vector.memset(ones_col[:, :], 1.0)
        p1 = pp.tile([1, 1], FDT, tag="pt")
        nc.tensor.matmul(p1[:, :], s1[:, :], ones_col[:, :], start=True, stop=True)
        nc.vector.tensor_copy(out=out_t[0:1, 0:1], in_=p1[0:1, 0:1])

        nc.sync.dma_start(out=part[:, :], in_=out_t[0:1, :])

    nc.compile()
    return nc


def _get_nc():
    if "nc" not in _CACHE:
        _CACHE["nc"] = _build()
    return _CACHE["nc"]


def _augment(xyz1_b, xyz2_b):
    """Host-side layout of one batch element into the augmented operands."""
    a1 = np.empty((9, N), dtype=np.float32)
    t1 = xyz1_b.T.astype(np.float32)           # [3, N]
    a1[0:3] = t1
    a1[3:6] = t1 * t1
    a1[6:9] = 1.0
    a2 = np.empty((9, M), dtype=np.float32)
    t2 = xyz2_b.T.astype(np.float32)           # [3, M]
    # negated so the PE emits -d2: all on-chip folds become MAX
    # (GPSIMD's partition_all_reduce implements max, not min)
    a2[0:3] = 2.0 * t2
    a2[3:6] = -1.0
    a2[6:9] = -(t2 * t2)
    return a1, a2


def run_cores(xyz1, xyz2, **kw):
    """Run the per-core kernel on all 8 cores; returns BassKernelResults."""
    xyz1 = np.asarray(xyz1, dtype=np.float32)
    xyz2 = np.asarray(xyz2, dtype=np.float32)
    assert xyz1.shape == (B, N, D) and xyz2.shape == (B, M, D)
    in_maps = []
    for b in range(B):
        a1, a2 = _augment(xyz1[b], xyz2[b])
        in_maps.append({"aug1": a1, "aug2": a2})
    return run_bass_kernel_spmd(_get_nc(), in_maps, list(range(B)), **kw)


def _combine(results):
    parts = np.stack([r["partial"][0] for r in results])  # [8, 2]
    s1 = float(parts[:, 0].astype(np.float64).sum())
    s2 = float(parts[:, 1].astype(np.float64).sum())
    return np.asarray(-(s1 / (B * N) + s2 / (B * M)), dtype=np.float32)


def kernel(xyz1, xyz2):
    res = run_cores(xyz1, xyz2)
    return _combine(res.results)
